# revision 9
# baseline (speedup 1.0000x reference)
"""Trainium2 Bass kernel for nn_CNNcond_9723805958518.

Computation (see reference.py): for embedded [B,S,D], filt [K*D,1], bias [1]:
    out[b, i] = sum_{k<K, d<D} embedded[b, i+k, d] * w[k, d] + bias
with K-1 zero frames padded past the end of the sequence.

Strategy (pure data parallelism over batch, 8 cores x 4 batches each):
  Stage 1 (TensorE): Y[k, j] = sum_d x[b, j, d] * w[k, d] as matmuls with
    d on the contraction partitions: lhsT = w^T [128, 16] per 128-d chunk,
    rhs = x^T [128, 512 positions], accumulating the 4 d-chunks in PSUM.
    x is pre-transposed to [D, S] per batch on the host so DMA loads are
    large contiguous reads (fp32 DMA-transpose is not available on trn2).
  Shift (DMA): out[i] needs sum_k Y[k, i+k] - a diagonal, which compute
    engines cannot address (no per-partition offsets). Y goes to a DRAM
    scratch with row pitch PITCH=S+16 and is read back with an access
    pattern of stride PITCH+1 per k-row, which lands Y[k, i+k] at [k, i].
    Row tails [S, PITCH) are pre-zeroed so reads past S see zeros.
  Stage 2 (TensorE): column-sum of the 16 aligned rows via a ones[16,1]
    matmul; bias is added during the PSUM->SBUF evacuation on ScalarE.

Precision modes:
  "bf16x3" (default): x and w are split on the host into bf16 hi+lo pairs
    (same total bytes as fp32) and stage 1 runs 3 bf16 passes
    xh*wh + xh*wl + xl*wh with fp32 PSUM accumulation; the dropped xl*wl
    term is ~2^-18 relative. Stage 2 splits Y into bf16 hi+lo at PSUM
    evacuation and sums both with ones-matmuls. ~1e-6 relative error at
    full PE rate (plain fp32 matmul runs 4 cycles/row and would be the
    bottleneck; float32r is full-rate but tf32-rounds to ~1.6e-4).
  "f32r": single-pass float32r matmuls (fastest PE, ~1.6e-4 rel err).
  "f32": plain fp32 matmuls (exact, PE-bound).
"""

import sys

import numpy as np

if "/opt/trn_rl_repo" not in sys.path:
    sys.path.append("/opt/trn_rl_repo")

import ml_dtypes

import concourse.bass as bass
import concourse.mybir as mybir
from concourse.bass_utils import run_bass_kernel_spmd
from concourse.tile import TileContext

# Problem constants (hardcoded per the harness contract).
B, S, D, K = 32, 4096, 512, 16
N_CORES = 8
BC = B // N_CORES  # batches per core
P = 128  # SBUF partitions / contraction size
DC = D // P  # d-chunks per position
TN = 512  # positions per matmul (PSUM bank = 512 fp32)
XH = 2048  # positions per x-tile load (SBUF budget)
NH = S // XH
NTH = XH // TN  # matmul tiles per x-tile
PITCH = S + K  # Y scratch row pitch
DIAG = PITCH + 1  # stride that walks the shifted diagonal
YFLAT = K * DIAG  # per-batch scratch elems (incl. rearrange pad)

_F32 = mybir.dt.float32
_BF16 = mybir.dt.bfloat16
BF = ml_dtypes.bfloat16

DEFAULT_MODE = "bf16x3"


def _split_multiwaits(nc, max_waits=1):
    """This container's walrus build accepts at most one sync-wait command
    per instruction ("Too many sync wait commands" in setupSyncWait
    otherwise). Splitting a multi-wait instruction into a chain of
    same-engine single-wait Drains is semantically identical: waits are
    conjunctive and each engine executes its stream in order."""
    n = 0
    for fn in nc.m.functions:
        for blk in fn.blocks:
            out = []
            for ins in blk.instructions:
                si = getattr(ins, "sync_info", None)
                waits = list(si.on_wait) if si is not None and si.on_wait else []
                if len(waits) > max_waits:
                    extra = waits[: len(waits) - max_waits]
                    si.on_wait = waits[len(waits) - max_waits :]
                    for i in range(0, len(extra), max_waits):
                        d = mybir.InstDrain(
                            name=nc.get_next_instruction_name(),
                            engine=ins.engine,
                            ins=[],
                            outs=[],
                            sync_info=mybir.SyncInfo(
                                on_wait=extra[i : i + max_waits], on_update=[]
                            ),
                        )
                        out.append(d)
                        n += 1
                out.append(ins)
            if len(out) != len(blk.instructions):
                blk.instructions = out
    return n


def build_nc_simple(mm_dt):
    """Single-pass variant: one x tensor / one w tensor of dtype mm_dt."""
    nc = bass.Bass("TRN2", debug=False)
    xt = nc.dram_tensor("xt", [BC, D, S], mm_dt, kind="ExternalInput")
    w = nc.dram_tensor("w", [P, DC * K], mm_dt, kind="ExternalInput")
    bias = nc.dram_tensor("bias", [1, 1], _F32, kind="ExternalInput")
    ones_d = nc.dram_tensor("ones", [K, 1], mm_dt, kind="ExternalInput")
    zer_d = nc.dram_tensor("zer", [K, K], mm_dt, kind="ExternalInput")
    out = nc.dram_tensor("out", [BC, S], _F32, kind="ExternalOutput")

    with TileContext(nc) as tc:
        with (
            tc.tile_pool(name="consts", bufs=1) as cpool,
            tc.tile_pool(name="xp", bufs=2) as xpool,
            tc.tile_pool(name="yp", bufs=2) as ypool,
            tc.tile_pool(name="afp", bufs=2) as apool,
            tc.tile_pool(name="obp", bufs=2) as opool,
            tc.tile_pool(name="psy", bufs=2, space="PSUM") as psy,
            tc.tile_pool(name="pso", bufs=2, space="PSUM") as pso,
            tc.tile_pool(name="dscr", bufs=1, space="DRAM") as dpool,
        ):
            wsb = cpool.tile([P, DC * K], mm_dt)
            nc.sync.dma_start(out=wsb[:, :], in_=w[:, :])
            bsb = cpool.tile([1, 1], _F32)
            nc.sync.dma_start(out=bsb[:, :], in_=bias[:, :])
            ones = cpool.tile([K, 1], mm_dt)
            nc.sync.dma_start(out=ones[:, :], in_=ones_d[:, :])
            zer = cpool.tile([K, K], mm_dt)
            nc.sync.dma_start(out=zer[:, :], in_=zer_d[:, :])
            yscr = dpool.tile([BC, YFLAT], mm_dt)

            for b in range(BC):
                tail = yscr[b, 0 : K * PITCH].rearrange("(k r) -> k r", r=PITCH)[
                    :, S:PITCH
                ]
                nc.sync.dma_start(out=tail, in_=zer[:, :])

            for b in range(BC):
                ybuf = ypool.tile([K, S], mm_dt)
                for h in range(NH):
                    xb = xpool.tile([P, DC * XH], mm_dt)
                    nc.sync.dma_start(
                        out=xb[:, :].rearrange("p (dc n) -> p dc n", n=XH),
                        in_=xt[b][:, h * XH : (h + 1) * XH].rearrange(
                            "(dc p) n -> p dc n", p=P
                        ),
                    )
                    for tt in range(NTH):
                        t = h * NTH + tt
                        py = psy.tile([K, TN], _F32)
                        for dc in range(DC):
                            nc.tensor.matmul(
                                py[:, :],
                                wsb[:, dc * K : (dc + 1) * K],
                                xb[:, dc * XH + tt * TN : dc * XH + (tt + 1) * TN],
                                start=(dc == 0),
                                stop=(dc == DC - 1),
                            )
                        nc.vector.tensor_copy(
                            ybuf[:, t * TN : (t + 1) * TN], py[:, :]
                        )

                ywr = yscr[b, 0 : K * PITCH].rearrange("(k r) -> k r", r=PITCH)[
                    :, 0:S
                ]
                nc.sync.dma_start(out=ywr, in_=ybuf[:, :])

                af = apool.tile([K, S], mm_dt)
                ard = yscr[b, :].rearrange("(k r) -> k r", r=DIAG)[:, 0:S]
                nc.sync.dma_start(out=af, in_=ard)

                ob = opool.tile([1, S], _F32)
                for t in range(S // TN):
                    po = pso.tile([1, TN], _F32)
                    nc.tensor.matmul(
                        po[:, :],
                        ones[:, :],
                        af[:, t * TN : (t + 1) * TN],
                        start=True,
                        stop=True,
                    )
                    nc.scalar.add(
                        ob[:, t * TN : (t + 1) * TN], po[:, :], bsb[0:1, 0:1]
                    )
                nc.sync.dma_start(out=out[b : b + 1, :], in_=ob[:, :])

    _split_multiwaits(nc)
    return nc


def build_nc_bf16x3(xh_=1024, xbufs=5):
    """3-pass bf16 split-precision variant (see module docstring).

    Pipelining details (from trace analysis of v1):
      - x is loaded in 1 MB chunks with deep buffering so the PE never
        starves long enough to trip the HAM re-throttle (~3.4 us).
      - DMA issue is spread over both HWDGE rings (nc.sync + nc.scalar);
        small const/zero DMAs go to SWDGE (nc.gpsimd) to keep them off
        the critical rings.
      - The Y scratch is split in two (cols [0, C1) and [R1, S+K)) so the
        first 3 stage-2 tiles run while stage 1 is still producing the
        second half, shrinking the serial per-batch tail.
      - Stage 2 sums hi and lo in ONE matmul over a stacked [32, *] tile.
    """
    xh = xh_
    nh = S // xh
    nth = xh // TN
    R1 = 3 * TN  # stage-2 split: tiles 0-2 from scratch 1
    C1 = R1 + TN  # scratch 1 holds Y cols [0, C1); needs C1 >= R1 + K
    W2 = S - R1 + K  # scratch 2 holds Y cols [R1, S) + K zero tail
    F1 = K * (C1 + 1)
    F2 = K * (W2 + 1)

    nc = bass.Bass("TRN2", debug=False)
    xth = nc.dram_tensor("xth", [BC, D, S], _BF16, kind="ExternalInput")
    xtl = nc.dram_tensor("xtl", [BC, D, S], _BF16, kind="ExternalInput")
    wd = nc.dram_tensor("w", [P, 2 * DC * K], _BF16, kind="ExternalInput")
    bias = nc.dram_tensor("bias", [1, 1], _F32, kind="ExternalInput")
    ones_d = nc.dram_tensor("ones", [2 * K, 1], _BF16, kind="ExternalInput")
    zer_d = nc.dram_tensor("zer", [K, K], _BF16, kind="ExternalInput")
    out = nc.dram_tensor("out", [BC, S], _F32, kind="ExternalOutput")

    with TileContext(nc) as tc:
        with (
            tc.tile_pool(name="consts", bufs=1) as cpool,
            tc.tile_pool(name="xph", bufs=xbufs) as xpool_h,
            tc.tile_pool(name="xpl", bufs=xbufs) as xpool_l,
            tc.tile_pool(name="yph", bufs=2) as ypool_h,
            tc.tile_pool(name="ypl", bufs=2) as ypool_l,
            tc.tile_pool(name="afp1", bufs=2) as apool1,
            tc.tile_pool(name="afp2", bufs=2) as apool2,
            tc.tile_pool(name="obp", bufs=2) as opool,
            tc.tile_pool(name="psy", bufs=3, space="PSUM") as psy,
            tc.tile_pool(name="pso", bufs=3, space="PSUM") as pso,
            tc.tile_pool(name="dscr", bufs=1, space="DRAM") as dpool,
        ):
            # w columns: [wh | wl], each [P, DC*K] with [p, dc*K+k].
            wsb = cpool.tile([P, 2 * DC * K], _BF16)
            nc.gpsimd.dma_start(out=wsb[:, :], in_=wd[:, :])
            bsb = cpool.tile([1, 1], _F32)
            nc.gpsimd.dma_start(out=bsb[:, :], in_=bias[:, :])
            ones = cpool.tile([2 * K, 1], _BF16)
            nc.gpsimd.dma_start(out=ones[:, :], in_=ones_d[:, :])
            zer = cpool.tile([K, K], _BF16)
            nc.gpsimd.dma_start(out=zer[:, :], in_=zer_d[:, :])
            # scr1[b]: rows of pitch C1 holding [Yh | Yl] cols [0, C1);
            # scr2[b]: rows of pitch W2 holding cols [R1, S) + zero tail.
            scr1_h = dpool.tile([BC, F1], _BF16)
            scr1_l = dpool.tile([BC, F1], _BF16)
            scr2_h = dpool.tile([BC, F2], _BF16)
            scr2_l = dpool.tile([BC, F2], _BF16)

            for b in range(BC):
                for scr in (scr2_h, scr2_l):
                    tail = scr[b, 0 : K * W2].rearrange("(k r) -> k r", r=W2)[
                        :, W2 - K : W2
                    ]
                    nc.gpsimd.dma_start(out=tail, in_=zer[:, :])

            for b in range(BC):
                ybh = ypool_h.tile([K, S], _BF16)
                ybl = ypool_l.tile([K, S], _BF16)
                ob = opool.tile([1, S], _F32)
                for h in range(nh):
                    xbh = xpool_h.tile([P, DC * xh], _BF16)
                    nc.sync.dma_start(
                        out=xbh[:, :].rearrange("p (dc n) -> p dc n", n=xh),
                        in_=xth[b][:, h * xh : (h + 1) * xh].rearrange(
                            "(dc p) n -> p dc n", p=P
                        ),
                    )
                    xbl = xpool_l.tile([P, DC * xh], _BF16)
                    nc.scalar.dma_start(
                        out=xbl[:, :].rearrange("p (dc n) -> p dc n", n=xh),
                        in_=xtl[b][:, h * xh : (h + 1) * xh].rearrange(
                            "(dc p) n -> p dc n", p=P
                        ),
                    )
                    for tt in range(nth):
                        t = h * nth + tt
                        py = psy.tile([K, TN], _F32)
                        first = True
                        # xh*wh + xl*wh + xh*wl; xl*wl (~2^-18 rel) dropped
                        for xb, woff in ((xbh, 0), (xbl, 0), (xbh, DC * K)):
                            for dc in range(DC):
                                nc.tensor.matmul(
                                    py[:, :],
                                    wsb[:, woff + dc * K : woff + (dc + 1) * K],
                                    xb[
                                        :,
                                        dc * xh + tt * TN : dc * xh + (tt + 1) * TN,
                                    ],
                                    start=first,
                                    stop=(xb is xbh and woff > 0 and dc == DC - 1),
                                )
                                first = False
                        # Evacuate as bf16 hi + lo: yh = bf16(py),
                        # yl = bf16(py - yh).
                        yhs = ybh[:, t * TN : (t + 1) * TN]
                        nc.vector.tensor_copy(yhs, py[:, :])
                        nc.vector.tensor_tensor(
                            ybl[:, t * TN : (t + 1) * TN],
                            py[:, :],
                            yhs,
                            mybir.AluOpType.subtract,
                        )
                        if (t + 1) * TN == C1:
                            # First C1 columns done: write scratch 1, read
                            # back the aligned diagonal, run early stage 2.
                            af1 = apool1.tile([2 * K, R1], _BF16)
                            for scr, yb, po_, eng in (
                                (scr1_h, ybh, 0, nc.sync),
                                (scr1_l, ybl, K, nc.scalar),
                            ):
                                eng.dma_start(
                                    out=scr[b, 0 : K * C1].rearrange(
                                        "(k r) -> k r", r=C1
                                    ),
                                    in_=yb[:, 0:C1],
                                )
                                eng.dma_start(
                                    out=af1[po_ : po_ + K, :],
                                    in_=scr[b, :].rearrange(
                                        "(k r) -> k r", r=C1 + 1
                                    )[:, 0:R1],
                                )
                            for t2 in range(R1 // TN):
                                po = pso.tile([1, TN], _F32)
                                nc.tensor.matmul(
                                    po[:, :],
                                    ones[:, :],
                                    af1[:, t2 * TN : (t2 + 1) * TN],
                                    start=True,
                                    stop=True,
                                )
                                nc.scalar.add(
                                    ob[:, t2 * TN : (t2 + 1) * TN],
                                    po[:, :],
                                    bsb[0:1, 0:1],
                                )

                # Remaining columns: scratch 2 (cols [R1, S) + zero tail).
                af2 = apool2.tile([2 * K, S - R1], _BF16)
                for scr, yb, po_, eng in (
                    (scr2_h, ybh, 0, nc.sync),
                    (scr2_l, ybl, K, nc.scalar),
                ):
                    eng.dma_start(
                        out=scr[b, 0 : K * W2].rearrange("(k r) -> k r", r=W2)[
                            :, 0 : S - R1
                        ],
                        in_=yb[:, R1:S],
                    )
                    eng.dma_start(
                        out=af2[po_ : po_ + K, :],
                        in_=scr[b, :].rearrange("(k r) -> k r", r=W2 + 1)[
                            :, 0 : S - R1
                        ],
                    )
                for t2 in range(R1 // TN, S // TN):
                    po = pso.tile([1, TN], _F32)
                    j = t2 * TN - R1
                    nc.tensor.matmul(
                        po[:, :],
                        ones[:, :],
                        af2[:, j : j + TN],
                        start=True,
                        stop=True,
                    )
                    nc.scalar.add(
                        ob[:, t2 * TN : (t2 + 1) * TN], po[:, :], bsb[0:1, 0:1]
                    )
                nc.scalar.dma_start(out=out[b : b + 1, :], in_=ob[:, :])

    _split_multiwaits(nc)
    return nc


_NC_CACHE = {}


def _get_nc(mode):
    if mode not in _NC_CACHE:
        if mode == "bf16x3":
            _NC_CACHE[mode] = build_nc_bf16x3()
        elif mode == "f32r":
            _NC_CACHE[mode] = build_nc_simple(mybir.dt.float32r)
        elif mode == "f32":
            _NC_CACHE[mode] = build_nc_simple(mybir.dt.float32)
        else:
            raise ValueError(mode)
    return _NC_CACHE[mode]


def _prep_in_maps(embedded, filt, bias, mode):
    embedded = np.ascontiguousarray(embedded, dtype=np.float32)
    filt = np.ascontiguousarray(filt, dtype=np.float32)
    bias = np.ascontiguousarray(bias, dtype=np.float32)
    b11 = bias.reshape(1, 1)

    def wl_layout(f):
        # [p, dc*K + k] = w[k, dc*128 + p]
        return np.ascontiguousarray(
            f.reshape(K, DC, P).transpose(2, 1, 0).reshape(P, DC * K)
        )

    in_maps = []
    if mode == "bf16x3":
        wh = filt.astype(BF)
        wlo = (filt - wh.astype(np.float32)).astype(BF)
        wcat = np.concatenate(
            [wl_layout(wh.astype(np.float32)), wl_layout(wlo.astype(np.float32))],
            axis=1,
        ).astype(BF)
        ones16 = np.ones((2 * K, 1), dtype=BF)
        zer16 = np.zeros((K, K), dtype=BF)
        xh = embedded.astype(BF)
        xl = (embedded - xh.astype(np.float32)).astype(BF)
        for c in range(N_CORES):
            sl = slice(c * BC, (c + 1) * BC)
            xthc = np.ascontiguousarray(xh[sl].transpose(0, 2, 1))
            xtlc = np.ascontiguousarray(xl[sl].transpose(0, 2, 1))
            in_maps.append(
                {
                    "xth": xthc,
                    "xtl": xtlc,
                    "w": wcat,
                    "bias": b11,
                    "ones": ones16,
                    "zer": zer16,
                }
            )
    else:
        wl = wl_layout(filt)
        ones16 = np.ones((K, 1), dtype=np.float32)
        zer16 = np.zeros((K, K), dtype=np.float32)
        for c in range(N_CORES):
            xc = embedded[c * BC : (c + 1) * BC]
            xtc = np.ascontiguousarray(xc.transpose(0, 2, 1))
            in_maps.append(
                {"xt": xtc, "w": wl, "bias": b11, "ones": ones16, "zer": zer16}
            )
    return in_maps


def run(embedded, filt, bias, mode=DEFAULT_MODE, trace=False, **spmd_kwargs):
    nc = _get_nc(mode)
    in_maps = _prep_in_maps(embedded, filt, bias, mode)
    res = run_bass_kernel_spmd(
        nc, in_maps, list(range(N_CORES)), trace=trace, **spmd_kwargs
    )
    out = np.concatenate([res.results[c]["out"] for c in range(N_CORES)], axis=0)
    return out.astype(np.float32), res


def kernel(embedded, filt, bias):
    out, _ = run(embedded, filt, bias)
    return out


# revision 11
# speedup vs baseline: 1.1492x; 1.1492x over previous
"""Trainium2 Bass kernel for nn_CNNcond_9723805958518.

Computation (see reference.py): for embedded [B,S,D], filt [K*D,1], bias [1]:
    out[b, i] = sum_{k<K, d<D} embedded[b, i+k, d] * w[k, d] + bias
with K-1 zero frames padded past the end of the sequence.

Strategy (pure data parallelism over batch, 8 cores x 4 batches each):
  Stage 1 (TensorE): Y[k, j] = sum_d x[b, j, d] * w[k, d] as matmuls with
    d on the contraction partitions: lhsT = w^T [128, 16] per 128-d chunk,
    rhs = x^T [128, 512 positions], accumulating the 4 d-chunks in PSUM.
    x is pre-transposed to [D, S] per batch on the host so DMA loads are
    large contiguous reads (fp32 DMA-transpose is not available on trn2).
  Shift (DMA): out[i] needs sum_k Y[k, i+k] - a diagonal, which compute
    engines cannot address (no per-partition offsets). Y goes to a DRAM
    scratch with row pitch PITCH=S+16 and is read back with an access
    pattern of stride PITCH+1 per k-row, which lands Y[k, i+k] at [k, i].
    Row tails [S, PITCH) are pre-zeroed so reads past S see zeros.
  Stage 2 (TensorE): column-sum of the 16 aligned rows via a ones[16,1]
    matmul; bias is added during the PSUM->SBUF evacuation on ScalarE.

Precision modes:
  "bf16x3" (default): x and w are split on the host into bf16 hi+lo pairs
    (same total bytes as fp32) and stage 1 runs 3 bf16 passes
    xh*wh + xh*wl + xl*wh with fp32 PSUM accumulation; the dropped xl*wl
    term is ~2^-18 relative. Stage 2 splits Y into bf16 hi+lo at PSUM
    evacuation and sums both with ones-matmuls. ~1e-6 relative error at
    full PE rate (plain fp32 matmul runs 4 cycles/row and would be the
    bottleneck; float32r is full-rate but tf32-rounds to ~1.6e-4).
  "f32r": single-pass float32r matmuls (fastest PE, ~1.6e-4 rel err).
  "f32": plain fp32 matmuls (exact, PE-bound).
"""

import sys

import numpy as np

if "/opt/trn_rl_repo" not in sys.path:
    sys.path.append("/opt/trn_rl_repo")

import ml_dtypes

import concourse.bass as bass
import concourse.mybir as mybir
from concourse.bass_utils import run_bass_kernel_spmd
from concourse.tile import TileContext

# Problem constants (hardcoded per the harness contract).
B, S, D, K = 32, 4096, 512, 16
N_CORES = 8
BC = B // N_CORES  # batches per core
P = 128  # SBUF partitions / contraction size
DC = D // P  # d-chunks per position
TN = 512  # positions per matmul (PSUM bank = 512 fp32)
XH = 2048  # positions per x-tile load (SBUF budget)
NH = S // XH
NTH = XH // TN  # matmul tiles per x-tile
PITCH = S + K  # Y scratch row pitch
DIAG = PITCH + 1  # stride that walks the shifted diagonal
YFLAT = K * DIAG  # per-batch scratch elems (incl. rearrange pad)

_F32 = mybir.dt.float32
_BF16 = mybir.dt.bfloat16
BF = ml_dtypes.bfloat16

DEFAULT_MODE = "bf16x3"


def _split_multiwaits(nc, max_waits=1):
    """This container's walrus build accepts at most one sync-wait command
    per instruction ("Too many sync wait commands" in setupSyncWait
    otherwise). Splitting a multi-wait instruction into a chain of
    same-engine single-wait Drains is semantically identical: waits are
    conjunctive and each engine executes its stream in order."""
    n = 0
    for fn in nc.m.functions:
        for blk in fn.blocks:
            out = []
            for ins in blk.instructions:
                si = getattr(ins, "sync_info", None)
                waits = list(si.on_wait) if si is not None and si.on_wait else []
                if len(waits) > max_waits:
                    extra = waits[: len(waits) - max_waits]
                    si.on_wait = waits[len(waits) - max_waits :]
                    for i in range(0, len(extra), max_waits):
                        # EVENT_SEMAPHORE is a pure wait carrier (~20-50 ns);
                        # a Drain here would flush the engine pipeline (on
                        # TensorE that costs microseconds per occurrence).
                        d = mybir.InstEventSemaphore(
                            name=nc.get_next_instruction_name(),
                            engine=ins.engine,
                            ins=[],
                            outs=[],
                            sync_info=mybir.SyncInfo(
                                on_wait=extra[i : i + max_waits], on_update=[]
                            ),
                        )
                        out.append(d)
                        n += 1
                out.append(ins)
            if len(out) != len(blk.instructions):
                blk.instructions = out
    return n


def build_nc_simple(mm_dt):
    """Single-pass variant: one x tensor / one w tensor of dtype mm_dt."""
    nc = bass.Bass("TRN2", debug=False)
    xt = nc.dram_tensor("xt", [BC, D, S], mm_dt, kind="ExternalInput")
    w = nc.dram_tensor("w", [P, DC * K], mm_dt, kind="ExternalInput")
    bias = nc.dram_tensor("bias", [1, 1], _F32, kind="ExternalInput")
    ones_d = nc.dram_tensor("ones", [K, 1], mm_dt, kind="ExternalInput")
    zer_d = nc.dram_tensor("zer", [K, K], mm_dt, kind="ExternalInput")
    out = nc.dram_tensor("out", [BC, S], _F32, kind="ExternalOutput")

    with TileContext(nc) as tc:
        with (
            tc.tile_pool(name="consts", bufs=1) as cpool,
            tc.tile_pool(name="xp", bufs=2) as xpool,
            tc.tile_pool(name="yp", bufs=2) as ypool,
            tc.tile_pool(name="afp", bufs=2) as apool,
            tc.tile_pool(name="obp", bufs=2) as opool,
            tc.tile_pool(name="psy", bufs=2, space="PSUM") as psy,
            tc.tile_pool(name="pso", bufs=2, space="PSUM") as pso,
            tc.tile_pool(name="dscr", bufs=1, space="DRAM") as dpool,
        ):
            wsb = cpool.tile([P, DC * K], mm_dt)
            nc.sync.dma_start(out=wsb[:, :], in_=w[:, :])
            bsb = cpool.tile([1, 1], _F32)
            nc.sync.dma_start(out=bsb[:, :], in_=bias[:, :])
            ones = cpool.tile([K, 1], mm_dt)
            nc.sync.dma_start(out=ones[:, :], in_=ones_d[:, :])
            zer = cpool.tile([K, K], mm_dt)
            nc.sync.dma_start(out=zer[:, :], in_=zer_d[:, :])
            yscr = dpool.tile([BC, YFLAT], mm_dt)

            for b in range(BC):
                tail = yscr[b, 0 : K * PITCH].rearrange("(k r) -> k r", r=PITCH)[
                    :, S:PITCH
                ]
                nc.sync.dma_start(out=tail, in_=zer[:, :])

            for b in range(BC):
                ybuf = ypool.tile([K, S], mm_dt)
                for h in range(NH):
                    xb = xpool.tile([P, DC * XH], mm_dt)
                    nc.sync.dma_start(
                        out=xb[:, :].rearrange("p (dc n) -> p dc n", n=XH),
                        in_=xt[b][:, h * XH : (h + 1) * XH].rearrange(
                            "(dc p) n -> p dc n", p=P
                        ),
                    )
                    for tt in range(NTH):
                        t = h * NTH + tt
                        py = psy.tile([K, TN], _F32)
                        for dc in range(DC):
                            nc.tensor.matmul(
                                py[:, :],
                                wsb[:, dc * K : (dc + 1) * K],
                                xb[:, dc * XH + tt * TN : dc * XH + (tt + 1) * TN],
                                start=(dc == 0),
                                stop=(dc == DC - 1),
                            )
                        nc.vector.tensor_copy(
                            ybuf[:, t * TN : (t + 1) * TN], py[:, :]
                        )

                ywr = yscr[b, 0 : K * PITCH].rearrange("(k r) -> k r", r=PITCH)[
                    :, 0:S
                ]
                nc.sync.dma_start(out=ywr, in_=ybuf[:, :])

                af = apool.tile([K, S], mm_dt)
                ard = yscr[b, :].rearrange("(k r) -> k r", r=DIAG)[:, 0:S]
                nc.sync.dma_start(out=af, in_=ard)

                ob = opool.tile([1, S], _F32)
                for t in range(S // TN):
                    po = pso.tile([1, TN], _F32)
                    nc.tensor.matmul(
                        po[:, :],
                        ones[:, :],
                        af[:, t * TN : (t + 1) * TN],
                        start=True,
                        stop=True,
                    )
                    nc.scalar.add(
                        ob[:, t * TN : (t + 1) * TN], po[:, :], bsb[0:1, 0:1]
                    )
                nc.sync.dma_start(out=out[b : b + 1, :], in_=ob[:, :])

    _split_multiwaits(nc)
    return nc


def build_nc_bf16x3(xh_=1024, xbufs=5):
    """3-pass bf16 split-precision variant (see module docstring).

    Pipelining details (from trace analysis of v1):
      - x is loaded in 1 MB chunks with deep buffering so the PE never
        starves long enough to trip the HAM re-throttle (~3.4 us).
      - DMA issue is spread over both HWDGE rings (nc.sync + nc.scalar);
        small const/zero DMAs go to SWDGE (nc.gpsimd) to keep them off
        the critical rings.
      - The Y scratch is split in two (cols [0, C1) and [R1, S+K)) so the
        first 3 stage-2 tiles run while stage 1 is still producing the
        second half, shrinking the serial per-batch tail.
      - Stage 2 sums hi and lo in ONE matmul over a stacked [32, *] tile.
    """
    xh = xh_
    nh = S // xh
    nth = xh // TN
    R1 = 3 * TN  # stage-2 split: tiles 0-2 from scratch 1
    C1 = R1 + TN  # scratch 1 holds Y cols [0, C1); needs C1 >= R1 + K
    W2 = S - R1 + K  # scratch 2 holds Y cols [R1, S) + K zero tail
    F1 = K * (C1 + 1)
    F2 = K * (W2 + 1)

    nc = bass.Bass("TRN2", debug=False)
    xth = nc.dram_tensor("xth", [BC, D, S], _BF16, kind="ExternalInput")
    xtl = nc.dram_tensor("xtl", [BC, D, S], _BF16, kind="ExternalInput")
    wd = nc.dram_tensor("w", [P, 2 * DC * K], _BF16, kind="ExternalInput")
    bias = nc.dram_tensor("bias", [1, 1], _F32, kind="ExternalInput")
    ones_d = nc.dram_tensor("ones", [2 * K, 1], _BF16, kind="ExternalInput")
    zer_d = nc.dram_tensor("zer", [K, K], _BF16, kind="ExternalInput")
    out = nc.dram_tensor("out", [BC, S], _F32, kind="ExternalOutput")

    with TileContext(nc) as tc:
        with (
            tc.tile_pool(name="consts", bufs=1) as cpool,
            tc.tile_pool(name="xph", bufs=xbufs) as xpool_h,
            tc.tile_pool(name="xpl", bufs=xbufs) as xpool_l,
            tc.tile_pool(name="yph", bufs=2) as ypool_h,
            tc.tile_pool(name="ypl", bufs=2) as ypool_l,
            tc.tile_pool(name="afp1", bufs=2) as apool1,
            tc.tile_pool(name="afp2", bufs=2) as apool2,
            tc.tile_pool(name="obp", bufs=2) as opool,
            tc.tile_pool(name="psy", bufs=3, space="PSUM") as psy,
            tc.tile_pool(name="pso", bufs=3, space="PSUM") as pso,
            tc.tile_pool(name="dscr", bufs=1, space="DRAM") as dpool,
        ):
            # w columns: [wh | wl], each [P, DC*K] with [p, dc*K+k].
            wsb = cpool.tile([P, 2 * DC * K], _BF16)
            nc.gpsimd.dma_start(out=wsb[:, :], in_=wd[:, :])
            bsb = cpool.tile([1, 1], _F32)
            nc.gpsimd.dma_start(out=bsb[:, :], in_=bias[:, :])
            ones = cpool.tile([2 * K, 1], _BF16)
            nc.gpsimd.dma_start(out=ones[:, :], in_=ones_d[:, :])
            zer = cpool.tile([K, K], _BF16)
            nc.gpsimd.dma_start(out=zer[:, :], in_=zer_d[:, :])
            # scr1[b]: rows of pitch C1 holding [Yh | Yl] cols [0, C1);
            # scr2[b]: rows of pitch W2 holding cols [R1, S) + zero tail.
            scr1_h = dpool.tile([BC, F1], _BF16)
            scr1_l = dpool.tile([BC, F1], _BF16)
            scr2_h = dpool.tile([BC, F2], _BF16)
            scr2_l = dpool.tile([BC, F2], _BF16)

            for b in range(BC):
                for scr in (scr2_h, scr2_l):
                    tail = scr[b, 0 : K * W2].rearrange("(k r) -> k r", r=W2)[
                        :, W2 - K : W2
                    ]
                    nc.gpsimd.dma_start(out=tail, in_=zer[:, :])

            for b in range(BC):
                ybh = ypool_h.tile([K, S], _BF16)
                ybl = ypool_l.tile([K, S], _BF16)
                ob = opool.tile([1, S], _F32)
                for h in range(nh):
                    xbh = xpool_h.tile([P, DC * xh], _BF16)
                    nc.sync.dma_start(
                        out=xbh[:, :].rearrange("p (dc n) -> p dc n", n=xh),
                        in_=xth[b][:, h * xh : (h + 1) * xh].rearrange(
                            "(dc p) n -> p dc n", p=P
                        ),
                    )
                    xbl = xpool_l.tile([P, DC * xh], _BF16)
                    nc.scalar.dma_start(
                        out=xbl[:, :].rearrange("p (dc n) -> p dc n", n=xh),
                        in_=xtl[b][:, h * xh : (h + 1) * xh].rearrange(
                            "(dc p) n -> p dc n", p=P
                        ),
                    )
                    for tt in range(nth):
                        t = h * nth + tt
                        py = psy.tile([K, TN], _F32)
                        first = True
                        # xh*wh + xl*wh + xh*wl; xl*wl (~2^-18 rel) dropped
                        for xb, woff in ((xbh, 0), (xbl, 0), (xbh, DC * K)):
                            for dc in range(DC):
                                nc.tensor.matmul(
                                    py[:, :],
                                    wsb[:, woff + dc * K : woff + (dc + 1) * K],
                                    xb[
                                        :,
                                        dc * xh + tt * TN : dc * xh + (tt + 1) * TN,
                                    ],
                                    start=first,
                                    stop=(xb is xbh and woff > 0 and dc == DC - 1),
                                )
                                first = False
                        # Evacuate as bf16 hi + lo: yh = bf16(py),
                        # yl = bf16(py - yh).
                        yhs = ybh[:, t * TN : (t + 1) * TN]
                        nc.vector.tensor_copy(yhs, py[:, :])
                        nc.vector.tensor_tensor(
                            ybl[:, t * TN : (t + 1) * TN],
                            py[:, :],
                            yhs,
                            mybir.AluOpType.subtract,
                        )
                        if (t + 1) * TN == C1:
                            # First C1 columns done: write scratch 1, read
                            # back the aligned diagonal, run early stage 2.
                            af1 = apool1.tile([2 * K, R1], _BF16)
                            for scr, yb, po_, eng in (
                                (scr1_h, ybh, 0, nc.gpsimd),
                                (scr1_l, ybl, K, nc.gpsimd),
                            ):
                                eng.dma_start(
                                    out=scr[b, 0 : K * C1].rearrange(
                                        "(k r) -> k r", r=C1
                                    ),
                                    in_=yb[:, 0:C1],
                                )
                                eng.dma_start(
                                    out=af1[po_ : po_ + K, :],
                                    in_=scr[b, :].rearrange(
                                        "(k r) -> k r", r=C1 + 1
                                    )[:, 0:R1],
                                )
                            for t2 in range(R1 // TN):
                                po = pso.tile([1, TN], _F32)
                                nc.tensor.matmul(
                                    po[:, :],
                                    ones[:, :],
                                    af1[:, t2 * TN : (t2 + 1) * TN],
                                    start=True,
                                    stop=True,
                                )
                                nc.scalar.add(
                                    ob[:, t2 * TN : (t2 + 1) * TN],
                                    po[:, :],
                                    bsb[0:1, 0:1],
                                )

                # Remaining columns: scratch 2 (cols [R1, S) + zero tail).
                af2 = apool2.tile([2 * K, S - R1], _BF16)
                for scr, yb, po_, eng in (
                    (scr2_h, ybh, 0, nc.gpsimd),
                    (scr2_l, ybl, K, nc.gpsimd),
                ):
                    eng.dma_start(
                        out=scr[b, 0 : K * W2].rearrange("(k r) -> k r", r=W2)[
                            :, 0 : S - R1
                        ],
                        in_=yb[:, R1:S],
                    )
                    eng.dma_start(
                        out=af2[po_ : po_ + K, :],
                        in_=scr[b, :].rearrange("(k r) -> k r", r=W2 + 1)[
                            :, 0 : S - R1
                        ],
                    )
                for t2 in range(R1 // TN, S // TN):
                    po = pso.tile([1, TN], _F32)
                    j = t2 * TN - R1
                    nc.tensor.matmul(
                        po[:, :],
                        ones[:, :],
                        af2[:, j : j + TN],
                        start=True,
                        stop=True,
                    )
                    nc.scalar.add(
                        ob[:, t2 * TN : (t2 + 1) * TN], po[:, :], bsb[0:1, 0:1]
                    )
                nc.gpsimd.dma_start(out=out[b : b + 1, :], in_=ob[:, :])

    _split_multiwaits(nc)
    return nc


_NC_CACHE = {}


def _get_nc(mode):
    if mode not in _NC_CACHE:
        if mode == "bf16x3":
            _NC_CACHE[mode] = build_nc_bf16x3()
        elif mode == "f32r":
            _NC_CACHE[mode] = build_nc_simple(mybir.dt.float32r)
        elif mode == "f32":
            _NC_CACHE[mode] = build_nc_simple(mybir.dt.float32)
        else:
            raise ValueError(mode)
    return _NC_CACHE[mode]


def _prep_in_maps(embedded, filt, bias, mode):
    embedded = np.ascontiguousarray(embedded, dtype=np.float32)
    filt = np.ascontiguousarray(filt, dtype=np.float32)
    bias = np.ascontiguousarray(bias, dtype=np.float32)
    b11 = bias.reshape(1, 1)

    def wl_layout(f):
        # [p, dc*K + k] = w[k, dc*128 + p]
        return np.ascontiguousarray(
            f.reshape(K, DC, P).transpose(2, 1, 0).reshape(P, DC * K)
        )

    in_maps = []
    if mode == "bf16x3":
        wh = filt.astype(BF)
        wlo = (filt - wh.astype(np.float32)).astype(BF)
        wcat = np.concatenate(
            [wl_layout(wh.astype(np.float32)), wl_layout(wlo.astype(np.float32))],
            axis=1,
        ).astype(BF)
        ones16 = np.ones((2 * K, 1), dtype=BF)
        zer16 = np.zeros((K, K), dtype=BF)
        xh = embedded.astype(BF)
        xl = (embedded - xh.astype(np.float32)).astype(BF)
        for c in range(N_CORES):
            sl = slice(c * BC, (c + 1) * BC)
            xthc = np.ascontiguousarray(xh[sl].transpose(0, 2, 1))
            xtlc = np.ascontiguousarray(xl[sl].transpose(0, 2, 1))
            in_maps.append(
                {
                    "xth": xthc,
                    "xtl": xtlc,
                    "w": wcat,
                    "bias": b11,
                    "ones": ones16,
                    "zer": zer16,
                }
            )
    else:
        wl = wl_layout(filt)
        ones16 = np.ones((K, 1), dtype=np.float32)
        zer16 = np.zeros((K, K), dtype=np.float32)
        for c in range(N_CORES):
            xc = embedded[c * BC : (c + 1) * BC]
            xtc = np.ascontiguousarray(xc.transpose(0, 2, 1))
            in_maps.append(
                {"xt": xtc, "w": wl, "bias": b11, "ones": ones16, "zer": zer16}
            )
    return in_maps


def run(embedded, filt, bias, mode=DEFAULT_MODE, trace=False, **spmd_kwargs):
    nc = _get_nc(mode)
    in_maps = _prep_in_maps(embedded, filt, bias, mode)
    res = run_bass_kernel_spmd(
        nc, in_maps, list(range(N_CORES)), trace=trace, **spmd_kwargs
    )
    out = np.concatenate([res.results[c]["out"] for c in range(N_CORES)], axis=0)
    return out.astype(np.float32), res


def kernel(embedded, filt, bias):
    out, _ = run(embedded, filt, bias)
    return out
